# revision 28
# baseline (speedup 1.0000x reference)
"""Multi-head self-attention Trainium2 kernel (Bass/Tile), v6.

Problem: x:(8,256,32,32), 8 heads, head_dim=32, N=H*W=1024.
Sharding: data-parallel over batch B=8 -> one batch element per NeuronCore.

Per-core math (b fixed, X = x[b] as (C=256, N=1024)):
  q = Wq@X + bq ; k = Wk@X + bk ; v = Wv@X + bv      (per-pixel linear)
  S[n,m] = sum_d q[d,n]k[d,m] / sqrt(32)  (per head)
  P = softmax_m(S) ; O[d,n] = sum_m P[n,m] v[d,m] ; out = Wo@O + bo + X

Bias algebra (folded on host, exact):
  - bk contributes q^T bk, constant along softmax axis -> drops.
  - bq contributes (bq^T k_raw)[m]: folded as an augmented K-hat row
    u_h = Wk_h^T bq_h / sqrt32, matched by a ones-row in Q-hat.
  - bv contributes bv (softmax weights sum to 1) -> xpb = x + (Wo@bv + bo).
  - 1/sqrt(32) folded into Wq-hat and the u rows.

Layouts: engine APs must sit at base partitions {0,32,64}, forcing the
64-row head pitch and DMA-based esum/O remaps:
  Qh/Kh: 8 head-slabs (head h rows 64h..64h+33; Q row 32 = ones memset,
         K row 32 = u row), 4 SBUF tiles [128,1024] bf16.
  S^T per (head-pair, m-chunk, n-half): psum [128, 2x512]; exp on ACT ->
  E [128,1024] bf16 (64 serial exps = the ~65us ACT floor).
  VH[mc] [128, 8*33] bf16: 32 V^T cols + a ones col per head, so the AV
  matmul also accumulates the softmax denominator (psO rows 32/96) free.

Schedule (vs the 135.7us v1 baseline):
  - All matmul operands bf16; PE work ~158K cycles, ACT ~65us -- the two
    jointly pace a steady state of 2 exps per (pair, m-chunk).
  - Scores double-buffered in a 4-bank psum rotation; projections /
    V^T / broadcast / output-projection transients live in a separate
    2-bank aux slab so their allocation never stalls the score stream;
    remaining 2 banks accumulate the pair's AV.
  - All non-score PE work is chunked to <=4 matmuls and spread across the
    mc slots where the exp backlog absorbs it.
  - Per-pair softmax denominators (ESUM[p]), reciprocal_approx_fast, and
    a split broadcast matmul let the pair-2 normalization half run
    mid-stream; only pair 3's half + output projection kc=1 trail.
  - First-wave DMA issue is spread over sync/scalar/gpsimd sequencers;
    first scores fire from half-granular chunk-0 projection copies.
"""

import math

import numpy as np
import ml_dtypes

import concourse.bass as bass
import concourse.mybir as mybir
import concourse.tile as tile
from concourse import bacc
from concourse.bass_utils import run_bass_kernel_spmd

F32 = mybir.dt.float32
F32R = mybir.dt.float32r
BF16 = mybir.dt.bfloat16
EXP = mybir.ActivationFunctionType.Exp

NH = 8          # heads
HD = 32         # head dim
C = 256         # channels
N = 1024        # H*W
NCORES = 8

_NC = None
LAST_RESULTS = None


def _emit(tc, io):
    nc = tc.nc
    import contextlib

    ctx = contextlib.ExitStack()
    with ctx:
        pers = ctx.enter_context(tc.tile_pool(name="pers", bufs=1))
        etp = ctx.enter_context(tc.tile_pool(name="etp", bufs=5))
        psp = ctx.enter_context(tc.tile_pool(name="psp", bufs=2, space="PSUM"))

        def ptile(name, shape, dtype=F32):
            return pers.tile(shape, dtype, tag=name, name=name)

        # ---------------- persistent tiles / input DMA ----------------
        XB = [ptile(f"XB{i}", [128, N], BF16) for i in range(2)]
        WQ = [ptile(f"WQ{i}", [128, 512], BF16) for i in range(2)]
        WK = [ptile(f"WK{i}", [128, 512], BF16) for i in range(2)]
        WV = [ptile(f"WV{i}", [128, C], BF16) for i in range(2)]
        WO = [ptile(f"WO{i}", [128, C], BF16) for i in range(2)]
        XPB = [ptile(f"XPB{i}", [128, N]) for i in range(2)]
        OHP = [ptile(f"OHP{i}", [2, 128], F32R) for i in range(2)]
        # first-wave loads in criticality order, issue spread over engine
        # sequencers (each DIRECT2D descriptor-gen is ~0.6us, serial per
        # engine -- single-engine issue delays the last load by ~5us)
        nc.sync.dma_start(WQ[0][:], io["wq"][0:128, :])
        for i in range(2):
            nc.scalar.dma_start(WK[i][:], io["wk"][i * 128 : (i + 1) * 128, :])
        for i in range(2):
            nc.gpsimd.dma_start(WV[i][:], io["wv"][i * 128 : (i + 1) * 128, :])
        for i in range(2):
            nc.gpsimd.dma_start(OHP[i][:], io["oh"][2 * i : 2 * i + 2, :])
        for i in range(2):
            nc.sync.dma_start(XB[i][:], io["xb"][i * 128 : (i + 1) * 128, :])
        nc.sync.dma_start(WQ[1][:], io["wq"][128:256, :])

        # warm the ACT exp table right after the first weight DMA lands
        # (output is never read; input is just convenient loaded data)
        warm = ptile("warm", [1, 32])
        nc.scalar.activation(warm[:], WQ[0][0:1, 0:32], EXP)

        Qh = [ptile(f"Qh{t}", [128, N], BF16) for t in range(4)]
        Kh = [ptile(f"Kh{t}", [128, N], BF16) for t in range(4)]
        VH = [ptile(f"VH{mc}", [128, NH * 33], BF16) for mc in range(NH)]
        O1U = [ptile(f"O1U{t}", [128, N]) for t in range(2)]
        ESUM = [ptile(f"ESUM{p}", [2, N]) for p in range(4)]
        O1 = [ptile(f"O1{t}", [128, N], BF16) for t in range(2)]
        PART = [ptile(f"PART{i}", [128, N]) for i in range(2)]
        OUTF = [ptile(f"OUTF{i}", [128, N]) for i in range(2)]

        # ---------------- building blocks ----------------
        def qk_pp(t, w, tag, bufs, nm):
            """Allocate the projection psum for one Q-hat/K-hat chunk and
            return per-half emitters (2 matmuls + trailing bf16 copy)."""
            pp = psp.tile([128, N], F32, tag=tag, bufs=bufs, name=f"pp{nm}{t}")
            dst = Qh if nm == "q" else Kh

            def half(jn):
                js = slice(jn * 512, (jn + 1) * 512)
                for kc in range(2):
                    nc.tensor.matmul(
                        pp[:, js],
                        w[kc][:, t * 128 : (t + 1) * 128],
                        XB[kc][:, js],
                        start=(kc == 0),
                        stop=(kc == 1),
                    )
                nc.vector.tensor_copy(dst[t][:, js], pp[:, js])
                if nm == "q":
                    for s in range(2):
                        nc.vector.memset(Qh[t][64 * s + 32 : 64 * s + 33, js], 1.0)

            return half

        def qk_units(t):
            """4 deferred units (Qh half0/1, Kh half0/1) for chunk t>0."""
            state = {}

            def unit(nm, w, jn):
                def run():
                    if nm not in state:
                        state[nm] = qk_pp(t, w, "aux", 1, nm)
                    state[nm](jn)

                return run

            return [
                unit("q", WQ, 0),
                unit("q", WQ, 1),
                unit("k", WK, 0),
                unit("k", WK, 1),
            ]

        def pv_pack(i):
            """V^T for m-chunks 2i and 2i+1 in one 1-bank psum alloc."""
            pvp = psp.tile([128, 512], F32, tag="aux", bufs=1, name=f"pvp{i}")
            for k in range(2):
                mc = 2 * i + k
                for kc in range(2):
                    nc.tensor.matmul(
                        pvp[:, k * 256 : (k + 1) * 256],
                        XB[kc][:, mc * 128 : (mc + 1) * 128],
                        WV[kc][:],
                        start=(kc == 0),
                        stop=(kc == 1),
                    )
            for k in range(2):
                mc = 2 * i + k
                nc.vector.memset(VH[mc][:], 1.0)
                vh3 = VH[mc].rearrange("p (h c) -> p h c", c=33)
                nc.vector.tensor_copy(
                    vh3[:, :, 0:32],
                    pvp[:, k * 256 : (k + 1) * 256].rearrange(
                        "p (h d) -> p h d", d=32
                    ),
                )

        def score_mms(p, mc, jn):
            ps = psp.tile([128, N], F32, tag="ps", bufs=2, name=f"ps{p}_{mc}_{jn}")
            for hh in range(2):
                base = 64 * hh
                nc.tensor.matmul(
                    ps[:, hh * 512 : (hh + 1) * 512],
                    Kh[p][base : base + 33, mc * 128 : (mc + 1) * 128],
                    Qh[p][base : base + 33, jn * 512 : (jn + 1) * 512],
                    start=True,
                    stop=True,
                )
            return ps

        def exp_op(p, mc, jn, ps):
            et = etp.tile([128, N], BF16, tag="et", name=f"et{p}_{mc}_{jn}")
            nc.scalar.activation(et[:], ps[:], EXP)
            return et

        def av_mms(p, mc, jn, et, psO):
            for hh in range(2):
                h = 2 * p + hh
                nc.tensor.matmul(
                    psO[jn][64 * hh : 64 * hh + 33, :],
                    VH[mc][:, 33 * h : 33 * h + 33],
                    et[:, hh * 512 : (hh + 1) * 512],
                    start=(mc == 0),
                    stop=(mc == 7),
                    tile_position=(0, 64 * hh),
                    skip_group_check=True,
                )

        def pair_out(p, psO):
            """Copy psO to SBUF; DMA O rows into O1U and esum rows into
            ESUM[p].  Esum rows go first (they gate the recip); for the last
            pair the copies run on ACT -- idle once the exps are done -- and
            the DMA issue is spread over the sync/gpsimd sequencers."""
            t = p // 2
            copy = nc.scalar.copy if p == 3 else nc.vector.tensor_copy
            for jn in range(2):
                js = slice(jn * 512, (jn + 1) * 512)
                ost = etp.tile([97, 512], F32, tag="ost", bufs=4, name=f"ost{p}_{jn}")
                copy(ost[32:33, :], psO[jn][32:33, :])
                copy(ost[64:97, :], psO[jn][64:97, :])
                for hh in range(2):
                    nc.sync.dma_start(
                        ESUM[p][hh : hh + 1, js],
                        ost[64 * hh + 32 : 64 * hh + 33, :],
                    )
                copy(ost[0:33, :], psO[jn][0:33, :])
                for hh in range(2):
                    h = 2 * p + hh
                    r = 32 * (h % 4)
                    nc.gpsimd.dma_start(
                        O1U[t][r : r + 32, js], ost[64 * hh : 64 * hh + 32, :]
                    )

        prn = {}

        def norm_half(t, parity):
            """recip + broadcast-matmul contribution of pair 2t+parity into
            pr[t].  The pair-even half of tile 1 runs mid-stream so only the
            pair-odd half is left in the tail chain."""
            p = 2 * t + parity
            if t not in prn:
                prn[t] = psp.tile([128, N], F32, tag="aux", bufs=1, name=f"prn{t}")
            RECIP = etp.tile([2, N], F32, tag="recip", bufs=2, name=f"recip{p}")
            RECIPR = etp.tile([2, N], F32R, tag="recipr", bufs=2, name=f"recipr{p}")
            for jn in range(2):
                js = slice(jn * 512, (jn + 1) * 512)
                with nc.allow_low_precision("softmax denom recip (~2e-6 rel)"):
                    nc.vector.reciprocal_approx_fast(RECIP[:, js], ESUM[p][:, js])
                nc.vector.tensor_copy(RECIPR[:, js], RECIP[:, js])
                nc.tensor.matmul(
                    prn[t][:, js],
                    OHP[parity][:],
                    RECIPR[:, js],
                    start=(parity == 0),
                    stop=(parity == 1),
                    skip_group_check=True,
                )

        def norm_finish(t, interleave_po=False):
            """O1 = O1U * pr; optionally chase each n-half with its kc=1
            output projection (tail pipelining)."""
            for jn in range(2):
                js = slice(jn * 512, (jn + 1) * 512)
                nc.vector.tensor_mul(O1[t][:, js], O1U[t][:, js], prn[t][:, js])
                if interleave_po:
                    for mo in range(2):
                        po_unit(1, mo, jn, "ps")

        def po_unit(kc, mo, jn, tag):
            js = slice(jn * 512, (jn + 1) * 512)
            po = psp.tile(
                [128, 512], F32, tag=tag, bufs=1 if tag == "aux" else 2,
                name=f"po{kc}_{mo}_{jn}",
            )
            nc.tensor.matmul(
                po[:],
                WO[kc][:, mo * 128 : (mo + 1) * 128],
                O1[kc][:, js],
                start=True,
                stop=True,
            )
            if kc == 0:
                nc.vector.tensor_add(PART[mo][:, js], po[:], XPB[mo][:, js])
            else:
                nc.vector.tensor_add(OUTF[mo][:, js], po[:], PART[mo][:, js])
                nc.gpsimd.dma_start(
                    io["out"][mo * 128 : (mo + 1) * 128, js], OUTF[mo][:, js]
                )

        # ---------------- emission schedule ----------------
        def new_psO(p):
            return [
                psp.tile([97, 512], F32, tag="psO", bufs=2, name=f"psO{p}_{jn}")
                for jn in range(2)
            ]

        psO = new_psO(0)

        # chunk 0 projections upfront on the (free) score-psum rotation, as
        # 1-bank per-half allocs interleaved with the first score matmuls so
        # the first exp fires as soon as the h0 copies land
        def qk0_half(w, nm, jn):
            pp = psp.tile([128, 512], F32, tag="ps", bufs=2, name=f"pp0{nm}{jn}")
            dst = Qh if nm == "q" else Kh
            js = slice(jn * 512, (jn + 1) * 512)
            for kc in range(2):
                nc.tensor.matmul(
                    pp[:], w[kc][:, 0:128], XB[kc][:, js],
                    start=(kc == 0), stop=(kc == 1),
                )
            nc.vector.tensor_copy(dst[0][:, js], pp[:])
            if nm == "q":
                for s in range(2):
                    nc.vector.memset(Qh[0][64 * s + 32 : 64 * s + 33, js], 1.0)

        qk0_half(WQ, "q", 0)
        qk0_half(WK, "k", 0)
        ps000 = score_mms(0, 0, 0)
        qk0_half(WQ, "q", 1)
        qk0_half(WK, "k", 1)
        ps001 = score_mms(0, 0, 1)

        qk1, qk2, qk3 = qk_units(1), qk_units(2), qk_units(3)
        late_dma = lambda: [
            (
                nc.sync.dma_start(WO[i][:], io["wo"][i * 128 : (i + 1) * 128, :]),
                nc.sync.dma_start(XPB[i][:], io["xpb"][i * 128 : (i + 1) * 128, :]),
            )
            for i in range(2)
        ]
        deferred = {
            (0, 0): [lambda: pv_pack(0)],
            (0, 1): [lambda: pv_pack(1)],
            (0, 2): [lambda: pv_pack(2)],
            (0, 3): [lambda: pv_pack(3)],
            (0, 4): [qk1[0]],
            (0, 5): [qk1[1]],
            (0, 6): [qk1[2]],
            (0, 7): [qk1[3]],
            (1, 0): [qk2[0]],
            (1, 1): [qk2[1]],
            (1, 2): [qk2[2]],
            (1, 3): [qk2[3]],
            (1, 4): [late_dma, qk3[0]],
            (1, 5): [qk3[1]],
            (1, 6): [qk3[2]],
            (1, 7): [qk3[3]],
            (2, 1): [lambda: norm_half(0, 0)],
            (2, 2): [lambda: (norm_half(0, 1), norm_finish(0))],
            (2, 3): [lambda: po_unit(0, 0, 0, "aux")],
            (3, 1): [lambda: norm_half(1, 0)],
            (2, 4): [lambda: po_unit(0, 1, 0, "aux")],
            (2, 5): [lambda: po_unit(0, 0, 1, "aux")],
            (2, 6): [lambda: po_unit(0, 1, 1, "aux")],
        }

        for p in range(4):
            if p > 0:
                psO = new_psO(p)
            for mc in range(8):
                if (p, mc) == (0, 0):
                    pss = [ps000, ps001]
                else:
                    pss = [score_mms(p, mc, jn) for jn in range(2)]
                ets = []
                for jn in range(2):
                    ets.append(exp_op(p, mc, jn, pss[jn]))
                for fn in deferred.get((p, mc), ()):
                    fn()
                for jn in range(2):
                    av_mms(p, mc, jn, ets[jn], psO)
            pair_out(p, psO)
        norm_half(1, 1)
        norm_finish(1, interleave_po=True)


def build_nc():
    nc = bacc.Bacc("TRN2", target_bir_lowering=False, debug=False)
    io = {}
    for name, shape, dt_ in [
        ("xb", (C, N), BF16),
        ("wq", (C, 512), BF16),
        ("wk", (C, 512), BF16),
        ("wv", (C, C), BF16),
        ("wo", (C, C), BF16),
        ("xpb", (C, N), F32),
        ("oh", (4, 128), F32R),
    ]:
        io[name] = nc.dram_tensor(name, shape, dt_, kind="ExternalInput").ap()
    io["out"] = nc.dram_tensor("out", (C, N), F32, kind="ExternalOutput").ap()
    with tile.TileContext(nc) as tc:
        _emit(tc, io)
    nc.finalize()
    return nc


def host_prep(x, Wq, bq, Wk, bk, Wv, bv, Wo, bo):
    """Build per-core input maps (numpy only)."""
    bf16 = ml_dtypes.bfloat16
    x = np.ascontiguousarray(np.asarray(x, np.float32))
    Wq, bq = np.asarray(Wq, np.float32), np.asarray(bq, np.float32)
    Wk = np.asarray(Wk, np.float32)
    Wv, bv = np.asarray(Wv, np.float32), np.asarray(bv, np.float32)
    Wo, bo = np.asarray(Wo, np.float32), np.asarray(bo, np.float32)
    s = 1.0 / math.sqrt(HD)

    wq_hat = np.zeros((C, 512), np.float32)
    wk_hat = np.zeros((C, 512), np.float32)
    for h in range(NH):
        hs = slice(HD * h, HD * (h + 1))
        wq_hat[:, 64 * h : 64 * h + 32] = Wq[hs, :].T * s
        wk_hat[:, 64 * h : 64 * h + 32] = Wk[hs, :].T
        wk_hat[:, 64 * h + 32] = (Wk[hs, :].T @ bq[hs]) * s

    # oh rows 0-1: pair-even stationary (recip row r -> O1U rows 32r..);
    # rows 2-3: pair-odd stationary (-> O1U rows 64+32r..)
    oh = np.zeros((4, 128), np.float32)
    for parity in range(2):
        for r in range(2):
            oh[2 * parity + r, 64 * parity + 32 * r : 64 * parity + 32 * r + 32] = 1.0

    bo2 = Wo @ bv + bo

    common = {
        "wq": wq_hat.astype(bf16),
        "wk": wk_hat.astype(bf16),
        "wv": np.ascontiguousarray(Wv.T).astype(bf16),
        "wo": np.ascontiguousarray(Wo.T).astype(bf16),
        "oh": oh,
    }

    B = x.shape[0]
    in_maps = []
    for b in range(B):
        xb = np.ascontiguousarray(x[b].reshape(C, N))
        m = dict(common)
        m["xb"] = xb.astype(bf16)
        m["xpb"] = np.ascontiguousarray(xb + bo2[:, None])
        in_maps.append(m)
    return in_maps


def kernel(x, Wq, bq, Wk, bk, Wv, bv, Wo, bo):
    global _NC, LAST_RESULTS
    if _NC is None:
        _NC = build_nc()
    in_maps = host_prep(x, Wq, bq, Wk, bk, Wv, bv, Wo, bo)
    res = run_bass_kernel_spmd(_NC, in_maps, core_ids=list(range(NCORES)))
    LAST_RESULTS = res
    out = np.stack([r["out"] for r in res.results], axis=0)
    return out.reshape(NCORES, C, 32, 32).astype(np.float32)


if __name__ == "__main__":
    rng = np.random.default_rng(0)
    ins = {
        "x": rng.standard_normal((8, C, 32, 32), dtype=np.float32),
        "Wq": rng.standard_normal((C, C), dtype=np.float32) / 16,
        "bq": rng.standard_normal(C).astype(np.float32) * 0.01,
        "Wk": rng.standard_normal((C, C), dtype=np.float32) / 16,
        "bk": rng.standard_normal(C).astype(np.float32) * 0.01,
        "Wv": rng.standard_normal((C, C), dtype=np.float32) / 16,
        "bv": rng.standard_normal(C).astype(np.float32) * 0.01,
        "Wo": rng.standard_normal((C, C), dtype=np.float32) / 16,
        "bo": rng.standard_normal(C).astype(np.float32) * 0.01,
    }
    out = kernel(**ins)
    print("out", out.shape, out.dtype, float(np.abs(out).mean()))
